# revision 8
# baseline (speedup 1.0000x reference)
"""Bahdanau-attention kernel for Trainium2, SPMD over 8 NeuronCores.

Math (per batch element b):
    c      = hidden[b] @ W_h + b_attn                  # (H,)
    z      = encoder[b] @ W_e                          # (T, H)
    energy = tanh(z + c)                               # (T, H)
    a      = energy @ v_w                              # (T,)
    w      = softmax(a)                                # (T,)
    ctx    = w @ encoder[b]                            # (E,)
Outputs: (context (B,E) f32, weights (B,T) f32).

Strategy: data-parallel over B across 8 cores (8 rows each). The host
pre-transposes encoder to (E, T) per batch element so the contraction dim E
lands on SBUF partitions with fully contiguous DMA (the PE array contracts
along partitions). On-chip, everything is computed in the "transposed"
orientation (h on partitions, t on free):
  - PE: z^T = W_e^T @ enc^T as 4 accumulated K=128 matmuls per (h-chunk, t-tile)
  - ACT: energy^T = tanh(z^T + c) with the bias as a per-partition vector
  - PE: logits = v^T @ energy^T (M=1 matvec, PSUM-accumulated over h-chunks)
  - ACT: exp() straight out of PSUM with fused free-dim accumulation (softmax
    denominator); no max-subtraction needed (|logits| <~ 6 for this problem)
  - DVE: ctx = sum_t w(t) * enc^T(e, t) via fused tensor_tensor_reduce at
    2x bf16 rate, chained across t-tiles
All matmul inputs are cast f32->bf16 during DMA (SWDGE).
"""

import numpy as np
from contextlib import ExitStack

from concourse import bass, bacc, mybir, tile
from concourse.bass_utils import run_bass_kernel_spmd
from concourse.dve_ops import TENSOR_TENSOR_REDUCE

B, T, H, E = 64, 4096, 512, 512
NCORES = 8
B_LOC = B // NCORES
EC = E // 128  # e-chunks
HC = H // 128  # h-chunks
TT_W = 1024    # t-macro width (2 psum banks)
NTT = T // TT_W
F32 = mybir.dt.float32
BF16 = mybir.dt.bfloat16


def build_nc(stage: int = 99):
    # stage: 1=MM1+tanh, 2=+logits/exp, 3=+softmax+w_out, 99=full
    nc = bacc.Bacc("TRN2", target_bir_lowering=False, debug=False)

    encT = nc.dram_tensor("encT", [B_LOC * E, T], F32, kind="ExternalInput").ap()
    hidT = nc.dram_tensor("hidT", [H, B_LOC], F32, kind="ExternalInput").ap()
    w_attn = nc.dram_tensor("w_attn", [H + E, H], F32, kind="ExternalInput").ap()
    b_r = nc.dram_tensor("b_r", [128, HC], F32, kind="ExternalInput").ap()
    v_r = nc.dram_tensor("v_r", [128, HC], F32, kind="ExternalInput").ap()

    ctx_raw = nc.dram_tensor("ctx_raw", [128, B_LOC * EC], F32, kind="ExternalOutput").ap()
    w_out = nc.dram_tensor("w_out", [B_LOC, T], F32, kind="ExternalOutput").ap()

    with tile.TileContext(nc) as tc, ExitStack() as ctx:
        const_pool = ctx.enter_context(tc.tile_pool(name="const", bufs=1))
        enc_pool = ctx.enter_context(tc.tile_pool(name="enc", bufs=3))
        energy_pool = ctx.enter_context(tc.tile_pool(name="energy", bufs=6))
        wb_pool = ctx.enter_context(tc.tile_pool(name="wb", bufs=2))
        scr_pool = ctx.enter_context(tc.tile_pool(name="scr", bufs=3))
        small_pool = ctx.enter_context(tc.tile_pool(name="small", bufs=2))
        zpsum = ctx.enter_context(tc.tile_pool(name="zpsum", bufs=2, space="PSUM"))
        apsum = ctx.enter_context(tc.tile_pool(name="apsum", bufs=2, space="PSUM"))
        cpsum = ctx.enter_context(tc.tile_pool(name="cpsum", bufs=1, space="PSUM"))

        # ---- constants / weights prep -------------------------------------
        we_sb = const_pool.tile([128, EC * H], BF16)   # [p, c*H + h] = W_e[c*128+p, h]
        nc.gpsimd.dma_start(
            out=we_sb[:].rearrange("p (c h) -> p c h", c=EC),
            in_=w_attn[H:].rearrange("(c p) h -> p c h", p=128)
        )
        wh_sb = const_pool.tile([128, EC * H], BF16)
        nc.gpsimd.dma_start(
            out=wh_sb[:].rearrange("p (c h) -> p c h", c=EC),
            in_=w_attn[:H].rearrange("(c p) h -> p c h", p=128)
        )
        hidt_sb = const_pool.tile([128, EC * B_LOC], BF16)
        nc.gpsimd.dma_start(
            out=hidt_sb[:].rearrange("p (c b) -> p c b", c=EC),
            in_=hidT.rearrange("(c p) b -> p c b", p=128)
        )
        b_sb = const_pool.tile([128, HC], F32)
        nc.gpsimd.dma_start(out=b_sb[:], in_=b_r)
        v_sb = const_pool.tile([128, HC], BF16)
        nc.gpsimd.dma_start(out=v_sb[:], in_=v_r)

        # c(h, b) = sum_e W_h[e, h] * hidden[b, e] + b_attn[h]
        c_sb = const_pool.tile([128, HC * B_LOC], F32)
        for k in range(HC):
            pc = cpsum.tile([128, B_LOC], F32)
            for c in range(EC):
                nc.tensor.matmul(
                    pc[:],
                    lhsT=wh_sb[:, c * H + k * 128 : c * H + (k + 1) * 128],
                    rhs=hidt_sb[:, c * B_LOC : (c + 1) * B_LOC],
                    start=(c == 0),
                    stop=(c == EC - 1),
                )
            nc.scalar.activation(
                out=c_sb[:, k * B_LOC : (k + 1) * B_LOC],
                in_=pc[:],
                func=mybir.ActivationFunctionType.Identity,
                bias=b_sb[:, k : k + 1],
            )

        ctx_all = const_pool.tile([128, B_LOC * EC], F32)

        # ---- main loop over local batch -----------------------------------
        for b in range(B_LOC):
            enc_sb = enc_pool.tile([128, EC * T], BF16)  # [p, c*T + t]
            nc.gpsimd.dma_start(
                out=enc_sb[:].rearrange("p (c t) -> p c t", c=EC),
                in_=encT[b * E : (b + 1) * E].rearrange("(c p) t -> p c t", p=128),
            )

            exp_b = wb_pool.tile([1, T], BF16, tag="exp")
            zp = small_pool.tile([1, 2 * NTT], F32, tag="zp")

            energies = {}
            for tt in range(NTT):
                t0 = tt * TT_W
                for k in range(HC):
                    pz = zpsum.tile([128, TT_W], F32)
                    for half in range(2):
                        s0 = half * 512
                        for c in range(EC):
                            nc.tensor.matmul(
                                pz[:, s0 : s0 + 512],
                                lhsT=we_sb[:, c * H + k * 128 : c * H + (k + 1) * 128],
                                rhs=enc_sb[:, c * T + t0 + s0 : c * T + t0 + s0 + 512],
                                start=(c == 0),
                                stop=(c == EC - 1),
                            )
                    en = energy_pool.tile([128, TT_W], BF16)
                    nc.scalar.activation(
                        out=en[:],
                        in_=pz[:],
                        func=mybir.ActivationFunctionType.Tanh,
                        bias=c_sb[:, k * B_LOC + b : k * B_LOC + b + 1],
                    )
                    energies[k] = en
                for half in range(2):
                    if stage < 2:
                        break
                    s0 = half * 512
                    pa = apsum.tile([1, 512], F32)
                    for k in range(HC):
                        nc.tensor.matmul(
                            pa[:],
                            lhsT=v_sb[:, k : k + 1],
                            rhs=energies[k][:, s0 : s0 + 512],
                            start=(k == 0),
                            stop=(k == HC - 1),
                        )
                    nc.scalar.activation(
                        out=exp_b[0:1, t0 + s0 : t0 + s0 + 512],
                        in_=pa[:],
                        func=mybir.ActivationFunctionType.Exp,
                        accum_out=zp[0:1, 2 * tt + half : 2 * tt + half + 1],
                    )

            # softmax denominator -> 1/Z, broadcast, scale
            if stage < 3:
                continue
            zb = small_pool.tile([1, 1], F32, tag="zb")
            nc.vector.tensor_reduce(
                out=zb[:], in_=zp[:], axis=mybir.AxisListType.X, op=mybir.AluOpType.add
            )
            rz = small_pool.tile([1, 1], F32, tag="rz")
            nc.vector.reciprocal(rz[:], zb[:])
            rz128 = small_pool.tile([128, 1], F32, tag="rz128")
            nc.gpsimd.partition_broadcast(rz128[:], rz[:])
            wbt = wb_pool.tile([128, T], BF16, tag="wb")
            nc.gpsimd.partition_broadcast(wbt[:], exp_b[:])
            wbs = wb_pool.tile([128, T], BF16, tag="wbs")
            nc.vector.tensor_scalar_mul(wbs[:], wbt[:], rz128[:, 0:1])

            # weights output row
            nc.gpsimd.dma_start(out=w_out[b : b + 1, :], in_=wbs[0:1, :])

            # ctx(e) = sum_t wbs(t) * encT(e, t): one fused multiply-reduce
            # per e-chunk (custom DVE op; the raw TENSOR_TENSOR_REDUCE ISA
            # opcode crashes this firmware)
            if stage < 4:
                continue
            for c in range(EC):
                col = ctx_all[:, b * EC + c : b * EC + c + 1]
                scr = scr_pool.tile([128, T], BF16)
                nc.vector._custom_dve(
                    TENSOR_TENSOR_REDUCE,
                    out=scr[:],
                    in0=enc_sb[:, c * T : (c + 1) * T],
                    in1=wbs[:],
                    s0=0.0,
                    s1=1.0,
                    accum_out=col,
                )

        nc.gpsimd.dma_start(out=ctx_raw, in_=ctx_all[:])

    nc.compile()
    return nc


_NC_CACHE = None


def _get_nc():
    global _NC_CACHE
    if _NC_CACHE is None:
        _NC_CACHE = build_nc()
    return _NC_CACHE


def _make_in_maps(inputs):
    hidden = np.asarray(inputs["hidden"], dtype=np.float32)
    encoder_outputs = np.asarray(inputs["encoder_outputs"], dtype=np.float32)
    W_attn = np.asarray(inputs["W_attn"], dtype=np.float32)
    b_attn = np.asarray(inputs["b_attn"], dtype=np.float32)
    v_w = np.asarray(inputs["v_w"], dtype=np.float32)

    b_r = np.ascontiguousarray(b_attn.reshape(HC, 128).T)  # [p, k] = b_attn[128k+p]
    v_r = np.ascontiguousarray(v_w.reshape(HC, 128).T)

    in_maps = []
    for i in range(NCORES):
        sl = slice(i * B_LOC, (i + 1) * B_LOC)
        encT = np.ascontiguousarray(
            encoder_outputs[sl].transpose(0, 2, 1)
        ).reshape(B_LOC * E, T)
        hidT = np.ascontiguousarray(hidden[sl].T)
        in_maps.append(
            {"encT": encT, "hidT": hidT, "w_attn": W_attn, "b_r": b_r, "v_r": v_r}
        )
    return in_maps


def kernel(hidden, encoder_outputs, W_attn, b_attn, v_w):
    in_maps = _make_in_maps(
        dict(
            hidden=hidden,
            encoder_outputs=encoder_outputs,
            W_attn=W_attn,
            b_attn=b_attn,
            v_w=v_w,
        )
    )
    nc = _get_nc()
    res = run_bass_kernel_spmd(nc, in_maps, core_ids=list(range(NCORES)))

    context = np.empty((B, E), dtype=np.float32)
    weights = np.empty((B, T), dtype=np.float32)
    for i, r in enumerate(res.results):
        sl = slice(i * B_LOC, (i + 1) * B_LOC)
        # ctx_raw[p, b*EC + c] = ctx[b, c*128 + p]
        cr = r["ctx_raw"].reshape(128, B_LOC, EC)
        context[sl] = cr.transpose(1, 2, 0).reshape(B_LOC, E)
        weights[sl] = r["w_out"]
    return context, weights


if __name__ == "__main__":
    rng = np.random.default_rng(0)
    out = kernel(
        hidden=rng.standard_normal((B, H), dtype=np.float32),
        encoder_outputs=rng.standard_normal((B, T, E), dtype=np.float32),
        W_attn=rng.standard_normal((H + E, H), dtype=np.float32) / 32.0,
        b_attn=rng.standard_normal((H,), dtype=np.float32) * 0.01,
        v_w=rng.standard_normal((H, 1), dtype=np.float32) / 32.0,
    )
    print("context", out[0].shape, "weights", out[1].shape)


# revision 10
# speedup vs baseline: 1.0580x; 1.0580x over previous
"""Bahdanau-attention kernel for Trainium2, SPMD over 8 NeuronCores.

Math (per batch element b):
    c      = hidden[b] @ W_h + b_attn                  # (H,)
    z      = encoder[b] @ W_e                          # (T, H)
    energy = tanh(z + c)                               # (T, H)
    a      = energy @ v_w                              # (T,)
    w      = softmax(a)                                # (T,)
    ctx    = w @ encoder[b]                            # (E,)
Outputs: (context (B,E) f32, weights (B,T) f32).

Strategy: data-parallel over B across 8 cores (8 rows each). The host
pre-transposes encoder to (E, T) per batch element so the contraction dim E
lands on SBUF partitions with fully contiguous DMA (the PE array contracts
along partitions). On-chip, everything is computed in the "transposed"
orientation (h on partitions, t on free):
  - PE: z^T = W_e^T @ enc^T as 4 accumulated K=128 matmuls per (h-chunk, t-tile)
  - ACT: energy^T = tanh(z^T + c) with the bias as a per-partition vector
  - PE: logits = v^T @ energy^T (M=1 matvec, PSUM-accumulated over h-chunks)
  - ACT: exp() straight out of PSUM with fused free-dim accumulation (softmax
    denominator); no max-subtraction needed (|logits| <~ 6 for this problem)
  - DVE: ctx = sum_t w(t) * enc^T(e, t) via fused tensor_tensor_reduce at
    2x bf16 rate, chained across t-tiles
All matmul inputs are cast f32->bf16 during DMA (SWDGE).
"""

import numpy as np
from contextlib import ExitStack

from concourse import bass, bacc, mybir, tile
from concourse.bass_utils import run_bass_kernel_spmd
from concourse.dve_ops import TENSOR_TENSOR_REDUCE

B, T, H, E = 64, 4096, 512, 512
NCORES = 8
B_LOC = B // NCORES
EC = E // 128  # e-chunks
HC = H // 128  # h-chunks
TT_W = 1024    # t-macro width (2 psum banks)
NTT = T // TT_W
F32 = mybir.dt.float32
BF16 = mybir.dt.bfloat16


def build_nc(stage: int = 99):
    # stage: 1=MM1+tanh, 2=+logits/exp, 3=+softmax+w_out, 99=full
    nc = bacc.Bacc("TRN2", target_bir_lowering=False, debug=False)

    encT = nc.dram_tensor("encT", [B_LOC * E, T], F32, kind="ExternalInput").ap()
    hidT = nc.dram_tensor("hidT", [H, B_LOC], F32, kind="ExternalInput").ap()
    w_attn = nc.dram_tensor("w_attn", [H + E, H], F32, kind="ExternalInput").ap()
    b_r = nc.dram_tensor("b_r", [128, HC], F32, kind="ExternalInput").ap()
    v_rep = nc.dram_tensor("v_rep", [128, H], F32, kind="ExternalInput").ap()

    ctx_raw = nc.dram_tensor("ctx_raw", [128, B_LOC * EC], F32, kind="ExternalOutput").ap()
    w_out = nc.dram_tensor("w_out", [B_LOC, T], F32, kind="ExternalOutput").ap()

    with tile.TileContext(nc) as tc, ExitStack() as ctx:
        const_pool = ctx.enter_context(tc.tile_pool(name="const", bufs=1))
        enc_pool = ctx.enter_context(tc.tile_pool(name="enc", bufs=3))
        energy_pool = ctx.enter_context(tc.tile_pool(name="energy", bufs=6))
        wb_pool = ctx.enter_context(tc.tile_pool(name="wb", bufs=2))
        scr_pool = ctx.enter_context(tc.tile_pool(name="scr", bufs=3))
        small_pool = ctx.enter_context(tc.tile_pool(name="small", bufs=2))
        zpsum = ctx.enter_context(tc.tile_pool(name="zpsum", bufs=3, space="PSUM"))
        apsum = ctx.enter_context(tc.tile_pool(name="apsum", bufs=2, space="PSUM"))

        # ---- constants / weights prep -------------------------------------
        we_sb = const_pool.tile([128, EC * H], BF16)   # [p, c*H + h] = W_e[c*128+p, h]
        nc.gpsimd.dma_start(
            out=we_sb[:].rearrange("p (c h) -> p c h", c=EC),
            in_=w_attn[H:].rearrange("(c p) h -> p c h", p=128)
        )
        wh_sb = const_pool.tile([128, EC * H], BF16)
        nc.gpsimd.dma_start(
            out=wh_sb[:].rearrange("p (c h) -> p c h", c=EC),
            in_=w_attn[:H].rearrange("(c p) h -> p c h", p=128)
        )
        hidt_sb = const_pool.tile([128, EC * B_LOC], BF16)
        nc.gpsimd.dma_start(
            out=hidt_sb[:].rearrange("p (c b) -> p c b", c=EC),
            in_=hidT.rearrange("(c p) b -> p c b", p=128)
        )
        b_sb = const_pool.tile([128, HC], F32)
        nc.gpsimd.dma_start(out=b_sb[:], in_=b_r)
        v_sb = const_pool.tile([128, H], BF16)
        nc.gpsimd.dma_start(out=v_sb[:], in_=v_rep)

        # c(h, b) = sum_e W_h[e, h] * hidden[b, e] + b_attn[h]
        c_sb = const_pool.tile([128, HC * B_LOC], F32)
        for k in range(HC):
            pc_full = apsum.tile([128, 512], F32, tag="a")
            pc = pc_full[:, :B_LOC]
            for c in range(EC):
                nc.tensor.matmul(
                    pc[:],
                    lhsT=wh_sb[:, c * H + k * 128 : c * H + (k + 1) * 128],
                    rhs=hidt_sb[:, c * B_LOC : (c + 1) * B_LOC],
                    start=(c == 0),
                    stop=(c == EC - 1),
                )
            nc.scalar.activation(
                out=c_sb[:, k * B_LOC : (k + 1) * B_LOC],
                in_=pc[:],
                func=mybir.ActivationFunctionType.Identity,
                bias=b_sb[:, k : k + 1],
            )

        ctx_all = const_pool.tile([128, B_LOC * EC], F32)

        # ---- main loop over local batch -----------------------------------
        for b in range(B_LOC):
            enc_sb = enc_pool.tile([128, EC * T], BF16)  # [p, c*T + t]
            for hlf in range(2):
                th = T // 2
                nc.gpsimd.dma_start(
                    out=enc_sb[:]
                    .rearrange("p (c t) -> p c t", c=EC)[:, :, hlf * th : (hlf + 1) * th],
                    in_=encT[b * E : (b + 1) * E].rearrange("(c p) t -> p c t", p=128)[
                        :, :, hlf * th : (hlf + 1) * th
                    ],
                )

            wbt = wb_pool.tile([128, T], BF16, tag="wb")
            zp = small_pool.tile([128, 2 * NTT], F32, tag="zp")

            energies = {}
            for tt in range(NTT):
                t0 = tt * TT_W
                for k in range(HC):
                    pz = zpsum.tile([128, TT_W], F32)
                    for half in range(2):
                        s0 = half * 512
                        for c in range(EC):
                            nc.tensor.matmul(
                                pz[:, s0 : s0 + 512],
                                lhsT=we_sb[:, c * H + k * 128 : c * H + (k + 1) * 128],
                                rhs=enc_sb[:, c * T + t0 + s0 : c * T + t0 + s0 + 512],
                                start=(c == 0),
                                stop=(c == EC - 1),
                            )
                    en = energy_pool.tile([128, TT_W], BF16)
                    nc.scalar.activation(
                        out=en[:],
                        in_=pz[:],
                        func=mybir.ActivationFunctionType.Tanh,
                        bias=c_sb[:, k * B_LOC + b : k * B_LOC + b + 1],
                    )
                    energies[k] = en
                for half in range(2):
                    if stage < 2:
                        break
                    s0 = half * 512
                    pa = apsum.tile([128, 512], F32, tag="a")
                    for k in range(HC):
                        nc.tensor.matmul(
                            pa[:],
                            lhsT=v_sb[:, k * 128 : (k + 1) * 128],
                            rhs=energies[k][:, s0 : s0 + 512],
                            start=(k == 0),
                            stop=(k == HC - 1),
                        )
                    nc.scalar.activation(
                        out=wbt[:, t0 + s0 : t0 + s0 + 512],
                        in_=pa[:],
                        func=mybir.ActivationFunctionType.Exp,
                        accum_out=zp[:, 2 * tt + half : 2 * tt + half + 1],
                    )

            # softmax denominator -> 1/Z, broadcast, scale
            if stage < 3:
                continue
            zb = small_pool.tile([128, 1], F32, tag="zb")
            nc.vector.tensor_reduce(
                out=zb[:], in_=zp[:], axis=mybir.AxisListType.X, op=mybir.AluOpType.add
            )
            rz128 = small_pool.tile([128, 1], F32, tag="rz128")
            nc.vector.reciprocal(rz128[:], zb[:])
            wbs = wb_pool.tile([128, T], BF16, tag="wbs")
            nc.vector.tensor_scalar_mul(wbs[:], wbt[:], rz128[:, 0:1])

            # weights output row
            nc.gpsimd.dma_start(out=w_out[b : b + 1, :], in_=wbs[0:1, :])

            # ctx(e) = sum_t wbs(t) * encT(e, t): one fused multiply-reduce
            # per e-chunk (custom DVE op; the raw TENSOR_TENSOR_REDUCE ISA
            # opcode crashes this firmware)
            if stage < 4:
                continue
            for c in range(EC):
                col = ctx_all[:, b * EC + c : b * EC + c + 1]
                scr = scr_pool.tile([128, T], BF16)
                nc.vector._custom_dve(
                    TENSOR_TENSOR_REDUCE,
                    out=scr[:],
                    in0=enc_sb[:, c * T : (c + 1) * T],
                    in1=wbt[:],
                    s0=0.0,
                    s1=rz128[:, 0:1],
                    accum_out=col,
                )

        nc.gpsimd.dma_start(out=ctx_raw, in_=ctx_all[:])

    nc.compile()
    return nc


_NC_CACHE = None


def _get_nc():
    global _NC_CACHE
    if _NC_CACHE is None:
        _NC_CACHE = build_nc()
    return _NC_CACHE


def _make_in_maps(inputs):
    hidden = np.asarray(inputs["hidden"], dtype=np.float32)
    encoder_outputs = np.asarray(inputs["encoder_outputs"], dtype=np.float32)
    W_attn = np.asarray(inputs["W_attn"], dtype=np.float32)
    b_attn = np.asarray(inputs["b_attn"], dtype=np.float32)
    v_w = np.asarray(inputs["v_w"], dtype=np.float32)

    b_r = np.ascontiguousarray(b_attn.reshape(HC, 128).T)  # [p, k] = b_attn[128k+p]
    # v_rep[p, 128k+m] = v[128k+p]  (column-replicated per h-chunk)
    v_rep = np.ascontiguousarray(
        np.repeat(v_w.reshape(HC, 128, 1), 128, axis=2).transpose(1, 0, 2).reshape(128, H)
    )

    in_maps = []
    for i in range(NCORES):
        sl = slice(i * B_LOC, (i + 1) * B_LOC)
        encT = np.ascontiguousarray(
            encoder_outputs[sl].transpose(0, 2, 1)
        ).reshape(B_LOC * E, T)
        hidT = np.ascontiguousarray(hidden[sl].T)
        in_maps.append(
            {"encT": encT, "hidT": hidT, "w_attn": W_attn, "b_r": b_r, "v_rep": v_rep}
        )
    return in_maps


def kernel(hidden, encoder_outputs, W_attn, b_attn, v_w):
    in_maps = _make_in_maps(
        dict(
            hidden=hidden,
            encoder_outputs=encoder_outputs,
            W_attn=W_attn,
            b_attn=b_attn,
            v_w=v_w,
        )
    )
    nc = _get_nc()
    res = run_bass_kernel_spmd(nc, in_maps, core_ids=list(range(NCORES)))

    context = np.empty((B, E), dtype=np.float32)
    weights = np.empty((B, T), dtype=np.float32)
    for i, r in enumerate(res.results):
        sl = slice(i * B_LOC, (i + 1) * B_LOC)
        # ctx_raw[p, b*EC + c] = ctx[b, c*128 + p]
        cr = r["ctx_raw"].reshape(128, B_LOC, EC)
        context[sl] = cr.transpose(1, 2, 0).reshape(B_LOC, E)
        weights[sl] = r["w_out"]
    return context, weights


if __name__ == "__main__":
    rng = np.random.default_rng(0)
    out = kernel(
        hidden=rng.standard_normal((B, H), dtype=np.float32),
        encoder_outputs=rng.standard_normal((B, T, E), dtype=np.float32),
        W_attn=rng.standard_normal((H + E, H), dtype=np.float32) / 32.0,
        b_attn=rng.standard_normal((H,), dtype=np.float32) * 0.01,
        v_w=rng.standard_normal((H, 1), dtype=np.float32) / 32.0,
    )
    print("context", out[0].shape, "weights", out[1].shape)


# revision 11
# speedup vs baseline: 1.0737x; 1.0149x over previous
"""Bahdanau-attention kernel for Trainium2, SPMD over 8 NeuronCores.

Math (per batch element b):
    c      = hidden[b] @ W_h + b_attn                  # (H,)
    z      = encoder[b] @ W_e                          # (T, H)
    energy = tanh(z + c)                               # (T, H)
    a      = energy @ v_w                              # (T,)
    w      = softmax(a)                                # (T,)
    ctx    = w @ encoder[b]                            # (E,)
Outputs: (context (B,E) f32, weights (B,T) f32).

Strategy: data-parallel over B across 8 cores (8 rows each). The host
pre-transposes encoder to (E, T) per batch element so the contraction dim E
lands on SBUF partitions with fully contiguous DMA (the PE array contracts
along partitions). On-chip, everything is computed in the "transposed"
orientation (h on partitions, t on free):
  - PE: z^T = W_e^T @ enc^T as 4 accumulated K=128 matmuls per (h-chunk, t-tile)
  - ACT: energy^T = tanh(z^T + c) with the bias as a per-partition vector
  - PE: logits = v^T @ energy^T (M=1 matvec, PSUM-accumulated over h-chunks)
  - ACT: exp() straight out of PSUM with fused free-dim accumulation (softmax
    denominator); no max-subtraction needed (|logits| <~ 6 for this problem)
  - DVE: ctx = sum_t w(t) * enc^T(e, t) via fused tensor_tensor_reduce at
    2x bf16 rate, chained across t-tiles
All matmul inputs are cast f32->bf16 during DMA (SWDGE).
"""

import numpy as np
from contextlib import ExitStack

from concourse import bass, bacc, mybir, tile
from concourse.bass_utils import run_bass_kernel_spmd
from concourse.dve_ops import TENSOR_TENSOR_REDUCE

B, T, H, E = 64, 4096, 512, 512
NCORES = 8
B_LOC = B // NCORES
EC = E // 128  # e-chunks
HC = H // 128  # h-chunks
TT_W = 1024    # t-macro width (2 psum banks)
NTT = T // TT_W
F32 = mybir.dt.float32
BF16 = mybir.dt.bfloat16


def build_nc(stage: int = 99):
    # stage: 1=MM1+tanh, 2=+logits/exp, 3=+softmax+w_out, 99=full
    nc = bacc.Bacc("TRN2", target_bir_lowering=False, debug=False)

    encT = nc.dram_tensor("encT", [B_LOC * E, T], F32, kind="ExternalInput").ap()
    hidT = nc.dram_tensor("hidT", [H, B_LOC], F32, kind="ExternalInput").ap()
    w_attn = nc.dram_tensor("w_attn", [H + E, H], F32, kind="ExternalInput").ap()
    b_r = nc.dram_tensor("b_r", [128, HC], F32, kind="ExternalInput").ap()
    v_rep = nc.dram_tensor("v_rep", [128, H], F32, kind="ExternalInput").ap()

    ctx_raw = nc.dram_tensor("ctx_raw", [128, B_LOC * EC], F32, kind="ExternalOutput").ap()
    w_out = nc.dram_tensor("w_out", [B_LOC, T], F32, kind="ExternalOutput").ap()

    with tile.TileContext(nc) as tc, ExitStack() as ctx:
        const_pool = ctx.enter_context(tc.tile_pool(name="const", bufs=1))
        enc_pool = ctx.enter_context(tc.tile_pool(name="enc", bufs=3))
        energy_pool = ctx.enter_context(tc.tile_pool(name="energy", bufs=10))
        wb_pool = ctx.enter_context(tc.tile_pool(name="wb", bufs=2))
        scr_pool = ctx.enter_context(tc.tile_pool(name="scr", bufs=3))
        small_pool = ctx.enter_context(tc.tile_pool(name="small", bufs=2))
        zpsum = ctx.enter_context(tc.tile_pool(name="zpsum", bufs=3, space="PSUM"))
        apsum = ctx.enter_context(tc.tile_pool(name="apsum", bufs=2, space="PSUM"))

        # ---- constants / weights prep -------------------------------------
        we_sb = const_pool.tile([128, EC * H], BF16)   # [p, c*H + h] = W_e[c*128+p, h]
        nc.gpsimd.dma_start(
            out=we_sb[:].rearrange("p (c h) -> p c h", c=EC),
            in_=w_attn[H:].rearrange("(c p) h -> p c h", p=128)
        )
        wh_sb = const_pool.tile([128, EC * H], BF16)
        nc.gpsimd.dma_start(
            out=wh_sb[:].rearrange("p (c h) -> p c h", c=EC),
            in_=w_attn[:H].rearrange("(c p) h -> p c h", p=128)
        )
        hidt_sb = const_pool.tile([128, EC * B_LOC], BF16)
        nc.gpsimd.dma_start(
            out=hidt_sb[:].rearrange("p (c b) -> p c b", c=EC),
            in_=hidT.rearrange("(c p) b -> p c b", p=128)
        )
        b_sb = const_pool.tile([128, HC], F32)
        nc.gpsimd.dma_start(out=b_sb[:], in_=b_r)
        v_sb = const_pool.tile([128, H], BF16)
        nc.gpsimd.dma_start(out=v_sb[:], in_=v_rep)

        # c(h, b) = sum_e W_h[e, h] * hidden[b, e] + b_attn[h]
        c_sb = const_pool.tile([128, HC * B_LOC], F32)
        for k in range(HC):
            pc_full = apsum.tile([128, 512], F32, tag="a")
            pc = pc_full[:, :B_LOC]
            for c in range(EC):
                nc.tensor.matmul(
                    pc[:],
                    lhsT=wh_sb[:, c * H + k * 128 : c * H + (k + 1) * 128],
                    rhs=hidt_sb[:, c * B_LOC : (c + 1) * B_LOC],
                    start=(c == 0),
                    stop=(c == EC - 1),
                )
            nc.scalar.activation(
                out=c_sb[:, k * B_LOC : (k + 1) * B_LOC],
                in_=pc[:],
                func=mybir.ActivationFunctionType.Identity,
                bias=b_sb[:, k : k + 1],
            )

        ctx_all = const_pool.tile([128, B_LOC * EC], F32)

        # ---- main loop over local batch -----------------------------------
        for b in range(B_LOC):
            enc_sb = enc_pool.tile([128, EC * T], BF16)  # [p, c*T + t]
            nparts = 4 if b == 0 else 2
            tp = T // nparts
            for prt in range(nparts):
                nc.gpsimd.dma_start(
                    out=enc_sb[:]
                    .rearrange("p (c t) -> p c t", c=EC)[:, :, prt * tp : (prt + 1) * tp],
                    in_=encT[b * E : (b + 1) * E].rearrange("(c p) t -> p c t", p=128)[
                        :, :, prt * tp : (prt + 1) * tp
                    ],
                )

            wbt = wb_pool.tile([128, T], BF16, tag="wb")
            zp = small_pool.tile([128, 2 * NTT], F32, tag="zp")

            energies = {}

            def logits_for(tt):
                t0 = tt * TT_W
                for half in range(2):
                    s0 = half * 512
                    pa = apsum.tile([128, 512], F32, tag="a")
                    for k in range(HC):
                        nc.tensor.matmul(
                            pa[:],
                            lhsT=v_sb[:, k * 128 : (k + 1) * 128],
                            rhs=energies[(tt, k)][:, s0 : s0 + 512],
                            start=(k == 0),
                            stop=(k == HC - 1),
                        )
                    nc.scalar.activation(
                        out=wbt[:, t0 + s0 : t0 + s0 + 512],
                        in_=pa[:],
                        func=mybir.ActivationFunctionType.Exp,
                        accum_out=zp[:, 2 * tt + half : 2 * tt + half + 1],
                    )

            for tt in range(NTT):
                t0 = tt * TT_W
                for k in range(HC):
                    pz = zpsum.tile([128, TT_W], F32)
                    for half in range(2):
                        s0 = half * 512
                        for c in range(EC):
                            nc.tensor.matmul(
                                pz[:, s0 : s0 + 512],
                                lhsT=we_sb[:, c * H + k * 128 : c * H + (k + 1) * 128],
                                rhs=enc_sb[:, c * T + t0 + s0 : c * T + t0 + s0 + 512],
                                start=(c == 0),
                                stop=(c == EC - 1),
                            )
                    en = energy_pool.tile([128, TT_W], BF16)
                    nc.scalar.activation(
                        out=en[:],
                        in_=pz[:],
                        func=mybir.ActivationFunctionType.Tanh,
                        bias=c_sb[:, k * B_LOC + b : k * B_LOC + b + 1],
                    )
                    energies[(tt, k)] = en
                if stage >= 2 and tt > 0:
                    logits_for(tt - 1)
            if stage >= 2:
                logits_for(NTT - 1)

            # softmax denominator -> 1/Z, broadcast, scale
            if stage < 3:
                continue
            zb = small_pool.tile([128, 1], F32, tag="zb")
            nc.vector.tensor_reduce(
                out=zb[:], in_=zp[:], axis=mybir.AxisListType.X, op=mybir.AluOpType.add
            )
            rz128 = small_pool.tile([128, 1], F32, tag="rz128")
            nc.vector.reciprocal(rz128[:], zb[:])
            wbs = wb_pool.tile([128, T], BF16, tag="wbs")
            nc.vector.tensor_scalar_mul(wbs[:], wbt[:], rz128[:, 0:1])

            # weights output row
            nc.gpsimd.dma_start(out=w_out[b : b + 1, :], in_=wbs[0:1, :])

            if stage < 4:
                continue
            TH = T // 2
            for c in range(EC):
                col = ctx_all[:, b * EC + c : b * EC + c + 1]
                for hlf in range(2):
                    scr = scr_pool.tile([128, TH], BF16)
                    nc.vector._custom_dve(
                        TENSOR_TENSOR_REDUCE,
                        out=scr[:],
                        in0=enc_sb[:, c * T + hlf * TH : c * T + hlf * TH + TH],
                        in1=wbt[:, hlf * TH : hlf * TH + TH],
                        s0=(0.0 if hlf == 0 else col),
                        s1=1.0,
                        accum_out=col,
                    )
            nc.vector.tensor_scalar_mul(
                ctx_all[:, b * EC : (b + 1) * EC],
                ctx_all[:, b * EC : (b + 1) * EC],
                rz128[:, 0:1],
            )



        nc.gpsimd.dma_start(out=ctx_raw, in_=ctx_all[:])

    nc.compile()
    return nc


_NC_CACHE = None


def _get_nc():
    global _NC_CACHE
    if _NC_CACHE is None:
        _NC_CACHE = build_nc()
    return _NC_CACHE


def _make_in_maps(inputs):
    hidden = np.asarray(inputs["hidden"], dtype=np.float32)
    encoder_outputs = np.asarray(inputs["encoder_outputs"], dtype=np.float32)
    W_attn = np.asarray(inputs["W_attn"], dtype=np.float32)
    b_attn = np.asarray(inputs["b_attn"], dtype=np.float32)
    v_w = np.asarray(inputs["v_w"], dtype=np.float32)

    b_r = np.ascontiguousarray(b_attn.reshape(HC, 128).T)  # [p, k] = b_attn[128k+p]
    # v_rep[p, 128k+m] = v[128k+p]  (column-replicated per h-chunk)
    v_rep = np.ascontiguousarray(
        np.repeat(v_w.reshape(HC, 128, 1), 128, axis=2).transpose(1, 0, 2).reshape(128, H)
    )

    in_maps = []
    for i in range(NCORES):
        sl = slice(i * B_LOC, (i + 1) * B_LOC)
        encT = np.ascontiguousarray(
            encoder_outputs[sl].transpose(0, 2, 1)
        ).reshape(B_LOC * E, T)
        hidT = np.ascontiguousarray(hidden[sl].T)
        in_maps.append(
            {"encT": encT, "hidT": hidT, "w_attn": W_attn, "b_r": b_r, "v_rep": v_rep}
        )
    return in_maps


def kernel(hidden, encoder_outputs, W_attn, b_attn, v_w):
    in_maps = _make_in_maps(
        dict(
            hidden=hidden,
            encoder_outputs=encoder_outputs,
            W_attn=W_attn,
            b_attn=b_attn,
            v_w=v_w,
        )
    )
    nc = _get_nc()
    res = run_bass_kernel_spmd(nc, in_maps, core_ids=list(range(NCORES)))

    context = np.empty((B, E), dtype=np.float32)
    weights = np.empty((B, T), dtype=np.float32)
    for i, r in enumerate(res.results):
        sl = slice(i * B_LOC, (i + 1) * B_LOC)
        # ctx_raw[p, b*EC + c] = ctx[b, c*128 + p]
        cr = r["ctx_raw"].reshape(128, B_LOC, EC)
        context[sl] = cr.transpose(1, 2, 0).reshape(B_LOC, E)
        weights[sl] = r["w_out"]
    return context, weights


if __name__ == "__main__":
    rng = np.random.default_rng(0)
    out = kernel(
        hidden=rng.standard_normal((B, H), dtype=np.float32),
        encoder_outputs=rng.standard_normal((B, T, E), dtype=np.float32),
        W_attn=rng.standard_normal((H + E, H), dtype=np.float32) / 32.0,
        b_attn=rng.standard_normal((H,), dtype=np.float32) * 0.01,
        v_w=rng.standard_normal((H, 1), dtype=np.float32) / 32.0,
    )
    print("context", out[0].shape, "weights", out[1].shape)


# revision 12
# speedup vs baseline: 1.1015x; 1.0259x over previous
"""Bahdanau-attention kernel for Trainium2, SPMD over 8 NeuronCores.

Math (per batch element b):
    c      = hidden[b] @ W_h + b_attn                  # (H,)
    z      = encoder[b] @ W_e                          # (T, H)
    energy = tanh(z + c)                               # (T, H)
    a      = energy @ v_w                              # (T,)
    w      = softmax(a)                                # (T,)
    ctx    = w @ encoder[b]                            # (E,)
Outputs: (context (B,E) f32, weights (B,T) f32).

Strategy: data-parallel over B across 8 cores (8 rows each). The host
pre-transposes encoder to (E, T) per batch element so the contraction dim E
lands on SBUF partitions with fully contiguous DMA (the PE array contracts
along partitions). On-chip, everything is computed in the "transposed"
orientation (h on partitions, t on free):
  - PE: z^T = W_e^T @ enc^T as 4 accumulated K=128 matmuls per (h-chunk, t-tile)
  - ACT: energy^T = tanh(z^T + c) with the bias as a per-partition vector
  - PE: logits = v^T @ energy^T (M=1 matvec, PSUM-accumulated over h-chunks)
  - ACT: exp() straight out of PSUM with fused free-dim accumulation (softmax
    denominator); no max-subtraction needed (|logits| <~ 6 for this problem)
  - DVE: ctx = sum_t w(t) * enc^T(e, t) via fused tensor_tensor_reduce at
    2x bf16 rate, chained across t-tiles
All matmul inputs are cast f32->bf16 during DMA (SWDGE).
"""

import numpy as np
from contextlib import ExitStack

from concourse import bass, bacc, mybir, tile
from concourse.bass_utils import run_bass_kernel_spmd
from concourse.dve_ops import TENSOR_TENSOR_REDUCE

B, T, H, E = 64, 4096, 512, 512
NCORES = 8
B_LOC = B // NCORES
EC = E // 128  # e-chunks
HC = H // 128  # h-chunks
TT_W = 1024    # t-macro width (2 psum banks)
NTT = T // TT_W
F32 = mybir.dt.float32
BF16 = mybir.dt.bfloat16


def build_nc(stage: int = 99):
    # stage: 1=MM1+tanh, 2=+logits/exp, 3=+softmax+w_out, 99=full
    nc = bacc.Bacc("TRN2", target_bir_lowering=False, debug=False)

    encT = nc.dram_tensor("encT", [B_LOC * E, T], F32, kind="ExternalInput").ap()
    hidT = nc.dram_tensor("hidT", [H, B_LOC], F32, kind="ExternalInput").ap()
    w_attn = nc.dram_tensor("w_attn", [H + E, H], F32, kind="ExternalInput").ap()
    b_r = nc.dram_tensor("b_r", [128, HC], F32, kind="ExternalInput").ap()
    v_rep = nc.dram_tensor("v_rep", [128, H], F32, kind="ExternalInput").ap()

    ctx_raw = nc.dram_tensor("ctx_raw", [128, B_LOC * EC], F32, kind="ExternalOutput").ap()
    w_out = nc.dram_tensor("w_out", [B_LOC, T], F32, kind="ExternalOutput").ap()

    with tile.TileContext(nc) as tc, ExitStack() as ctx:
        const_pool = ctx.enter_context(tc.tile_pool(name="const", bufs=1))
        enc_pool = ctx.enter_context(tc.tile_pool(name="enc", bufs=3))
        energy_pool = ctx.enter_context(tc.tile_pool(name="energy", bufs=10))
        wb_pool = ctx.enter_context(tc.tile_pool(name="wb", bufs=2))
        scr_pool = ctx.enter_context(tc.tile_pool(name="scr", bufs=3))
        small_pool = ctx.enter_context(tc.tile_pool(name="small", bufs=2))
        zpsum = ctx.enter_context(tc.tile_pool(name="zpsum", bufs=3, space="PSUM"))
        apsum = ctx.enter_context(tc.tile_pool(name="apsum", bufs=2, space="PSUM"))

        # ---- constants / weights prep -------------------------------------
        we_sb = const_pool.tile([128, EC * H], BF16)   # [p, c*H + h] = W_e[c*128+p, h]
        nc.gpsimd.dma_start(
            out=we_sb[:].rearrange("p (c h) -> p c h", c=EC),
            in_=w_attn[H:].rearrange("(c p) h -> p c h", p=128)
        )
        wh_sb = const_pool.tile([128, EC * H], BF16)
        nc.gpsimd.dma_start(
            out=wh_sb[:].rearrange("p (c h) -> p c h", c=EC),
            in_=w_attn[:H].rearrange("(c p) h -> p c h", p=128)
        )
        hidt_sb = const_pool.tile([128, EC * B_LOC], BF16)
        nc.gpsimd.dma_start(
            out=hidt_sb[:].rearrange("p (c b) -> p c b", c=EC),
            in_=hidT.rearrange("(c p) b -> p c b", p=128)
        )
        b_sb = const_pool.tile([128, HC], F32)
        nc.gpsimd.dma_start(out=b_sb[:], in_=b_r)
        v_sb = const_pool.tile([128, H], BF16)
        nc.gpsimd.dma_start(out=v_sb[:], in_=v_rep)

        # c(h, b) = sum_e W_h[e, h] * hidden[b, e] + b_attn[h]
        c_sb = const_pool.tile([128, HC * B_LOC], F32)
        for k in range(HC):
            pc_full = apsum.tile([128, 512], F32, tag="a")
            pc = pc_full[:, :B_LOC]
            for c in range(EC):
                nc.tensor.matmul(
                    pc[:],
                    lhsT=wh_sb[:, c * H + k * 128 : c * H + (k + 1) * 128],
                    rhs=hidt_sb[:, c * B_LOC : (c + 1) * B_LOC],
                    start=(c == 0),
                    stop=(c == EC - 1),
                )
            nc.scalar.activation(
                out=c_sb[:, k * B_LOC : (k + 1) * B_LOC],
                in_=pc[:],
                func=mybir.ActivationFunctionType.Identity,
                bias=b_sb[:, k : k + 1],
            )

        ctx_all = const_pool.tile([128, B_LOC * EC], F32)

        # ---- main loop over local batch -----------------------------------
        for b in range(B_LOC):
            enc_sb = enc_pool.tile([128, EC * T], BF16)  # [p, c*T + t]
            nparts = 8 if b == 0 else 2
            tp = T // nparts
            for prt in range(nparts):
                nc.gpsimd.dma_start(
                    out=enc_sb[:]
                    .rearrange("p (c t) -> p c t", c=EC)[:, :, prt * tp : (prt + 1) * tp],
                    in_=encT[b * E : (b + 1) * E].rearrange("(c p) t -> p c t", p=128)[
                        :, :, prt * tp : (prt + 1) * tp
                    ],
                )

            TH = T // 2
            wbt0 = wb_pool.tile([128, TH], BF16, tag="wb0")
            wbt1 = wb_pool.tile([128, TH], BF16, tag="wb1")
            wbt_half = [wbt0, wbt1]
            zp = small_pool.tile([128, 2 * NTT], F32, tag="zp")

            energies = {}

            def logits_for(tt):
                t0 = tt * TT_W
                for half in range(2):
                    s0 = half * 512
                    pa = apsum.tile([128, 512], F32, tag="a")
                    for k in range(HC):
                        nc.tensor.matmul(
                            pa[:],
                            lhsT=v_sb[:, k * 128 : (k + 1) * 128],
                            rhs=energies[(tt, k)][:, s0 : s0 + 512],
                            start=(k == 0),
                            stop=(k == HC - 1),
                        )
                    toff = t0 + s0
                    nc.scalar.activation(
                        out=wbt_half[toff // TH][:, toff % TH : toff % TH + 512],
                        in_=pa[:],
                        func=mybir.ActivationFunctionType.Exp,
                        accum_out=zp[:, 2 * tt + half : 2 * tt + half + 1],
                    )

            for tt in range(NTT):
                t0 = tt * TT_W
                for k in range(HC):
                    pz = zpsum.tile([128, TT_W], F32)
                    for half in range(2):
                        s0 = half * 512
                        for c in range(EC):
                            nc.tensor.matmul(
                                pz[:, s0 : s0 + 512],
                                lhsT=we_sb[:, c * H + k * 128 : c * H + (k + 1) * 128],
                                rhs=enc_sb[:, c * T + t0 + s0 : c * T + t0 + s0 + 512],
                                start=(c == 0),
                                stop=(c == EC - 1),
                            )
                    en = energy_pool.tile([128, TT_W], BF16)
                    nc.scalar.activation(
                        out=en[:],
                        in_=pz[:],
                        func=mybir.ActivationFunctionType.Tanh,
                        bias=c_sb[:, k * B_LOC + b : k * B_LOC + b + 1],
                    )
                    energies[(tt, k)] = en
                if stage >= 2 and tt > 0:
                    logits_for(tt - 1)
            if stage >= 2:
                logits_for(NTT - 1)

            # softmax denominator -> 1/Z, broadcast, scale
            if stage < 3:
                continue
            zb = small_pool.tile([128, 1], F32, tag="zb")
            nc.vector.tensor_reduce(
                out=zb[:], in_=zp[:], axis=mybir.AxisListType.X, op=mybir.AluOpType.add
            )
            rz128 = small_pool.tile([128, 1], F32, tag="rz128")
            nc.vector.reciprocal(rz128[:], zb[:])
            wbs = wb_pool.tile([128, T], BF16, tag="wbs")
            for hlf in range(2):
                nc.vector.tensor_scalar_mul(
                    wbs[:, hlf * TH : (hlf + 1) * TH], wbt_half[hlf][:], rz128[:, 0:1]
                )

            # weights output row
            nc.gpsimd.dma_start(out=w_out[b : b + 1, :], in_=wbs[0:1, :])

            if stage < 4:
                continue
            for c in range(EC):
                col = ctx_all[:, b * EC + c : b * EC + c + 1]
                for hlf in range(2):
                    scr = scr_pool.tile([128, TH], BF16)
                    nc.vector._custom_dve(
                        TENSOR_TENSOR_REDUCE,
                        out=scr[:],
                        in0=enc_sb[:, c * T + hlf * TH : c * T + hlf * TH + TH],
                        in1=wbt_half[hlf][:],
                        s0=(0.0 if hlf == 0 else col),
                        s1=1.0,
                        accum_out=col,
                    )
            nc.vector.tensor_scalar_mul(
                ctx_all[:, b * EC : (b + 1) * EC],
                ctx_all[:, b * EC : (b + 1) * EC],
                rz128[:, 0:1],
            )



        nc.gpsimd.dma_start(out=ctx_raw, in_=ctx_all[:])

    nc.compile()
    return nc


_NC_CACHE = None


def _get_nc():
    global _NC_CACHE
    if _NC_CACHE is None:
        _NC_CACHE = build_nc()
    return _NC_CACHE


def _make_in_maps(inputs):
    hidden = np.asarray(inputs["hidden"], dtype=np.float32)
    encoder_outputs = np.asarray(inputs["encoder_outputs"], dtype=np.float32)
    W_attn = np.asarray(inputs["W_attn"], dtype=np.float32)
    b_attn = np.asarray(inputs["b_attn"], dtype=np.float32)
    v_w = np.asarray(inputs["v_w"], dtype=np.float32)

    b_r = np.ascontiguousarray(b_attn.reshape(HC, 128).T)  # [p, k] = b_attn[128k+p]
    # v_rep[p, 128k+m] = v[128k+p]  (column-replicated per h-chunk)
    v_rep = np.ascontiguousarray(
        np.repeat(v_w.reshape(HC, 128, 1), 128, axis=2).transpose(1, 0, 2).reshape(128, H)
    )

    in_maps = []
    for i in range(NCORES):
        sl = slice(i * B_LOC, (i + 1) * B_LOC)
        encT = np.ascontiguousarray(
            encoder_outputs[sl].transpose(0, 2, 1)
        ).reshape(B_LOC * E, T)
        hidT = np.ascontiguousarray(hidden[sl].T)
        in_maps.append(
            {"encT": encT, "hidT": hidT, "w_attn": W_attn, "b_r": b_r, "v_rep": v_rep}
        )
    return in_maps


def kernel(hidden, encoder_outputs, W_attn, b_attn, v_w):
    in_maps = _make_in_maps(
        dict(
            hidden=hidden,
            encoder_outputs=encoder_outputs,
            W_attn=W_attn,
            b_attn=b_attn,
            v_w=v_w,
        )
    )
    nc = _get_nc()
    res = run_bass_kernel_spmd(nc, in_maps, core_ids=list(range(NCORES)))

    context = np.empty((B, E), dtype=np.float32)
    weights = np.empty((B, T), dtype=np.float32)
    for i, r in enumerate(res.results):
        sl = slice(i * B_LOC, (i + 1) * B_LOC)
        # ctx_raw[p, b*EC + c] = ctx[b, c*128 + p]
        cr = r["ctx_raw"].reshape(128, B_LOC, EC)
        context[sl] = cr.transpose(1, 2, 0).reshape(B_LOC, E)
        weights[sl] = r["w_out"]
    return context, weights


if __name__ == "__main__":
    rng = np.random.default_rng(0)
    out = kernel(
        hidden=rng.standard_normal((B, H), dtype=np.float32),
        encoder_outputs=rng.standard_normal((B, T, E), dtype=np.float32),
        W_attn=rng.standard_normal((H + E, H), dtype=np.float32) / 32.0,
        b_attn=rng.standard_normal((H,), dtype=np.float32) * 0.01,
        v_w=rng.standard_normal((H, 1), dtype=np.float32) / 32.0,
    )
    print("context", out[0].shape, "weights", out[1].shape)
